# revision 10
# baseline (speedup 1.0000x reference)
"""DFT-D3 (zero damping, static all-pairs) two-body dispersion energy on 8
Trainium2 NeuronCores.

Strategy (i-slab sharding): each core owns 64 atoms i and computes the
[512 j, 64 i, 27 s] pair tensor r2 with one bf16-split TensorE matmul per
512-column chunk.  All rationals run in log space with *tanh* in place of
sigmoid -- sigma(x) = (1+tanh(x/2))/2 -- so every activation the kernel
needs (ln, exp, tanh, square, copy) lives in just two activation-table
sets (natural_log_exp, exp_and_others), eliminating table thrash.

Key identities / choices:
  - g8 = (8/7) g6 + rho with rho = (1/7)ln6 + 16 ln(RS6/RS18): the r0ab
    dependence cancels, so the 8-term damping comes straight from g6 via
    activation scale/bias (no second STT pass).
  - Ln reads the matmul PSUM directly (no r2 clamp; host verifies
    min pair r2 is safely positive, else numpy fallback).
  - 6-term chain in bf16 (2x/4x DVE modes); tanh8 stays fp32 because
    (1+tanh8) catastrophically cancels against the huge r^-8 factor.
  - cn partial sums -> per-slab phi (c6 interpolation weights) computed
    locally, AllGather'ed early (overlapped with the remaining scalar
    work), so the post-collective tail is just Q/W2 matmuls + final dot.
  - s-reductions split between DVE and the Pool engine (gpsimd) to keep
    the Scalar (activation) engine the only near-saturated unit.
"""
import os
os.environ.setdefault("JAX_PLATFORMS", "cpu")

import math
import numpy as np
import ml_dtypes

import concourse.bass as bass
import concourse.mybir as mybir
from concourse.tile import TileContext
from concourse.bass_utils import run_bass_kernel_spmd

F32 = mybir.dt.float32
BF16 = mybir.dt.bfloat16
AF = mybir.ActivationFunctionType
OP = mybir.AluOpType

# D3 constants
AUTOANG = 0.52917726
AUTOEV = 27.21138505
K1, K3 = 16.0, -4.0
CUTOFF, CNTHR = 95.0, 40.0
S6, RS6, S18, RS18, ALP = 1.0, 1.217, 0.722, 1.0, 14.0

N = 512          # atoms
NS = 27          # lattice shifts
NCORES = 8
SLAB = N // NCORES           # 64 atoms per core
JB = 4                       # j blocks of 128
FREE = SLAB * NS             # 1728
NZ = 95                      # species table size
NG = 5                       # cn grid points per axis

SELF_R2 = 1.0e4              # value added to self pairs via matmul
R2MIN_HOST = 2.5e-4          # host-checked min pair r2 (no on-device clamp)
C14L = float(math.log(6.0) + 14.0 * math.log(RS6))   # ln(6*RS6^14)
RHO = float(math.log(6.0) / 7.0 + 16.0 * math.log(RS6 / RS18))
CB8 = float(3.0 * S18)
LN05 = float(math.log(0.5))

_CHUNKS = [(0, 512), (512, 1024), (1024, 1536), (1536, 1728)]

# packed-input column layouts
P95F_COLS = dict(r0abT=(0, NZ), ohZT=(NZ, NZ + N), ohZiT=(NZ + N, NZ + N + SLAB))
P95F_W = NZ + N + SLAB           # 671
P2F_W = N + SLAB                 # Lrco | Rrco
P1F_W = SLAB + NG * SLAB         # r2r4sl | GROW
LR_W = N + FREE                  # Lcat | Rcat


def _bc_s(ap2d, s=NS):
    """[128, M] AP -> [128, M, s] with stride-0 broadcast over s."""
    a3 = ap2d[:, :, None]
    new = [list(a3.ap[0]), list(a3.ap[1]), [0, s]]
    return bass.AP(a3.tensor, a3.offset, new)


def _split_excess_waits(nc, max_waits=1):
    """This walrus build accepts at most one sync wait per instruction;
    Tile's tail drain can carry several. Hoist excess waits onto inserted
    drains on the same engine (sequential waits == conjunction)."""
    n_split = 0
    for f in nc.m.functions:
        for b in f.blocks:
            new_list = []
            changed = False
            for ins in b.instructions:
                si = ins.sync_info
                if si is not None:
                    waits = list(si.on_wait or [])
                    updates = list(si.on_update or [])
                    if len(waits) > max_waits:
                        excess = waits[: len(waits) - max_waits]
                        keep = waits[len(waits) - max_waits:]
                        for w in excess:
                            d = mybir.InstDrain(
                                name=f"I-waitsplit-{n_split}", ins=[], outs=[])
                            n_split += 1
                            d.engine = ins.engine
                            d.sync_info = mybir.SyncInfo(on_wait=[w], on_update=[])
                            new_list.append(d)
                            changed = True
                        ins.sync_info = mybir.SyncInfo(
                            on_wait=list(keep), on_update=list(updates))
                new_list.append(ins)
            if changed:
                b.instructions = new_list
    return n_split


_orig_clear_sems = bass.Bass.clear_and_free_semaphores


def _chunked_clear_sems(self, sems, _chunk=4):
    """This walrus build rejects EVENT_SEMAPHORE_RANGE_CLEAR over wide
    ranges; clear in chunks of <=4."""
    nums = sorted(s.num if hasattr(s, "num") else s for s in sems)
    for i in range(0, len(nums), _chunk):
        _orig_clear_sems(self, nums[i:i + _chunk])


bass.Bass.clear_and_free_semaphores = _chunked_clear_sems


def build_program():
    nc = bass.Bass(num_devices=NCORES)

    LRcat = nc.dram_tensor("LRcat", [94, LR_W], BF16, kind="ExternalInput")
    P95F = nc.dram_tensor("P95F", [NZ, P95F_W], F32, kind="ExternalInput")
    P2F = nc.dram_tensor("P2F", [2, P2F_W], F32, kind="ExternalInput")
    P1F = nc.dram_tensor("P1F", [1, P1F_W], F32, kind="ExternalInput")
    R2R4C = nc.dram_tensor("R2R4C", [128, JB], F32, kind="ExternalInput")
    C2cat = nc.dram_tensor("C2cat", [NZ, 2 * NG * NZ * NG], BF16,
                           kind="ExternalInput")
    OHZB = nc.dram_tensor("OHZB", [128, JB * NZ], BF16, kind="ExternalInput")

    e_part = nc.dram_tensor("e_part", [SLAB], F32, kind="ExternalOutput")

    with TileContext(nc) as tc:
        with (
            tc.tile_pool(name="const", bufs=1) as cpool,
            tc.tile_pool(name="L2p", bufs=1) as l2pool,
            tc.tile_pool(name="g6p", bufs=1) as g6pool,
            tc.tile_pool(name="usp", bufs=1) as uspool,
            tc.tile_pool(name="rot", bufs=2) as tpool,
            tc.tile_pool(name="red", bufs=1) as rpool,
            tc.tile_pool(name="sm", bufs=2) as spool,
            tc.tile_pool(name="ph", bufs=1) as ppool,
            tc.tile_pool(name="psR", bufs=1, space="PSUM") as psR,
            tc.tile_pool(name="psS", bufs=1, space="PSUM") as psS,
            tc.tile_pool(name="psC", bufs=1, space="PSUM") as psC,
            tc.tile_pool(name="psW", bufs=1, space="PSUM") as psW,
            tc.tile_pool(name="psP", bufs=1, space="PSUM") as psP,
            tc.tile_pool(name="dram", bufs=1, space="DRAM") as dpool,
        ):
            # ---------- input DMAs ----------
            LR_s = cpool.tile([94, LR_W], BF16, tag="LRcat")
            nc.sync.dma_start(LR_s[:], LRcat[:])
            P95_s = cpool.tile([NZ, P95F_W], F32, tag="P95F")
            nc.sync.dma_start(P95_s[:], P95F[:])
            P2_s = cpool.tile([2, P2F_W], F32, tag="P2F")
            nc.sync.dma_start(P2_s[:], P2F[:])
            P1_s = cpool.tile([1, P1F_W], F32, tag="P1F")
            nc.sync.dma_start(P1_s[:], P1F[:])
            r2r4c_s = cpool.tile([128, JB], F32, tag="R2R4C")
            nc.sync.dma_start(r2r4c_s[:], R2R4C[:])
            # phase-2-only loads on the Pool SWDGE queue
            C2_s = cpool.tile([NZ, 2 * NG * NZ * NG], BF16, tag="C2cat")
            nc.gpsimd.dma_start(C2_s[:], C2cat[:])
            ohzb_s = cpool.tile([128, JB * NZ], BF16, tag="OHZB")
            nc.gpsimd.dma_start(ohzb_s[:], OHZB[:])

            ones128x1 = cpool.tile([128, 1], F32, tag="ones128")
            nc.gpsimd.memset(ones128x1[:], 1.0)
            ones1x95 = cpool.tile([1, NZ], F32, tag="ones95")
            nc.gpsimd.memset(ones1x95[:], 1.0)
            ones1x128 = cpool.tile([1, 128], F32, tag="ones1x128")
            nc.gpsimd.memset(ones1x128[:], 1.0)
            bias_k1 = cpool.tile([128, 1], F32, tag="bias_k1")
            nc.gpsimd.memset(bias_k1[:], -float(K1 / 2.0))
            bias_rho = cpool.tile([128, 1], F32, tag="bias_rho")
            nc.gpsimd.memset(bias_rho[:], float(RHO / 2.0))
            bias_ln05 = cpool.tile([128, 1], F32, tag="bias_ln05")
            nc.gpsimd.memset(bias_ln05[:], LN05)

            c0_, c1_ = P95F_COLS["r0abT"]
            r0abT_s = P95_s[:, c0_:c1_]
            ohZT0 = P95F_COLS["ohZT"][0]
            oi0, oi1 = P95F_COLS["ohZiT"]
            ohZiT_s = P95_s[:, oi0:oi1]
            r2r4sl_s = P1_s[:, 0:SLAB]
            GROW_s = P1_s[:, SLAB:SLAB + NG * SLAB]   # [1, 320] grid row

            # ---------- prep matmuls ----------
            R1_ps = psS.tile([NZ, SLAB], F32, tag="small")
            nc.tensor.matmul(R1_ps[:], r0abT_s, ohZiT_s, start=True, stop=True)
            R1_s = cpool.tile([NZ, SLAB], F32, tag="R1")
            nc.scalar.copy(R1_s[:], R1_ps[:])

            r2r4i_ps = psS.tile([128, SLAB], F32, tag="small")
            nc.tensor.matmul(r2r4i_ps[:], ones1x128[:], r2r4sl_s,
                             start=True, stop=True)
            r2r4i_s = cpool.tile([128, SLAB], F32, tag="r2r4i")
            nc.vector.tensor_copy(r2r4i_s[:], r2r4i_ps[:])

            lnc14_s, rco_s = [], []
            for b in range(JB):
                r0p_ps = psS.tile([128, SLAB], F32, tag="small")
                nc.tensor.matmul(r0p_ps[:],
                                 P95_s[:, ohZT0 + b * 128:ohZT0 + (b + 1) * 128],
                                 R1_s[:], start=True, stop=True)
                lr = spool.tile([128, SLAB], F32, tag="lnr0")
                nc.scalar.activation(lr[:], r0p_ps[:], AF.Ln)
                lnc14 = rpool.tile([128, SLAB], F32, tag=f"lnc14_{b}")
                nc.vector.tensor_scalar(lnc14[:], lr[:], 14.0, C14L,
                                        OP.mult, OP.add)
                lnc14_s.append(lnc14)

                rco_ps = psS.tile([128, SLAB], F32, tag="small")
                nc.tensor.matmul(rco_ps[:], P2_s[:, b * 128:(b + 1) * 128],
                                 P2_s[:, N:N + SLAB], start=True, stop=True)
                rco = rpool.tile([128, SLAB], F32, tag=f"rco_{b}")
                nc.vector.tensor_copy(rco[:], rco_ps[:])
                rco_s.append(rco)

            # ---------- phase 1: r2 -> L2 (table A), g6, cn chain ----------
            L2_s, g6_s, us_s = [], [], []
            for b in range(JB):
                jsl = slice(b * 128, (b + 1) * 128)
                r2_ps = psR.tile([128, FREE], F32, tag="r2ps")
                for (c0, c1) in _CHUNKS:
                    nc.tensor.matmul(r2_ps[:, c0:c1], LR_s[:, jsl],
                                     LR_s[:, N + c0:N + c1],
                                     start=True, stop=True)
                L2 = l2pool.tile([128, FREE], F32, tag=f"L2_{b}")
                nc.scalar.activation(L2[:], r2_ps[:], AF.Ln)
                L2_s.append(L2)
                # u05 in the same (ln/exp) table set
                us = uspool.tile([128, FREE], F32, tag=f"us_{b}")
                nc.scalar.activation(us[:], L2[:], AF.Exp, scale=-0.5)
                us_s.append(us)
                # g6 on DVE while Scalar continues
                g6 = g6pool.tile([128, FREE], F32, tag=f"g6_{b}")
                nc.vector.scalar_tensor_tensor(
                    g6[:].rearrange("p (i s) -> p i s", s=NS),
                    L2[:].rearrange("p (i s) -> p i s", s=NS),
                    7.0, _bc_s(lnc14_s[b][:]),
                    OP.mult, OP.subtract)
                g6_s.append(g6)
                # us *= rco  (in place, fp32) -- on Pool to offload DVE
                nc.gpsimd.tensor_tensor(
                    us[:].rearrange("p (i s) -> p i s", s=NS),
                    us[:].rearrange("p (i s) -> p i s", s=NS),
                    _bc_s(rco_s[b][:]), OP.mult)

            # ---------- cn chain (tanh, in place on us) ----------
            for b in range(JB):
                nc.scalar.activation(us_s[b][:], us_s[b][:], AF.Tanh,
                                     bias=bias_k1[:], scale=float(K1 / 2.0))
                cnred = rpool.tile([128, SLAB], F32, tag=f"cnred_{b}")
                nc.vector.tensor_reduce(
                    cnred[:], us_s[b][:].rearrange("p (i s) -> p i s", s=NS),
                    axis=mybir.AxisListType.X, op=OP.add)
                if b == 0:
                    cn_ps = psC.tile([1, SLAB], F32, tag="cnps")
                nc.tensor.matmul(cn_ps[:], ones128x1[:], cnred[:],
                                 start=(b == 0), stop=(b == JB - 1))

            # ---------- slab phi row [1, 320] + collective ----------
            cn_row = ppool.tile([1, SLAB], F32, tag="cn_row")
            # cn = 0.5 * sum(tanh) + 0.5 * (N * NS)
            nc.vector.tensor_scalar(cn_row[:], cn_ps[:], 0.5,
                                    float(0.5 * N * NS), OP.mult, OP.add)
            d_row = ppool.tile([1, NG * SLAB], F32, tag="d_row")
            cn_bc = bass.AP(cn_row[:].tensor, cn_row[:].offset,
                            [list(cn_row[:].ap[0]), [0, NG], [1, SLAB]])
            nc.vector.tensor_tensor(
                d_row[:].rearrange("p (g k) -> p g k", g=NG),
                cn_bc,
                GROW_s.rearrange("p (g k) -> p g k", g=NG), OP.subtract)
            sq_row = ppool.tile([1, NG * SLAB], F32, tag="sq_row")
            nc.scalar.activation(sq_row[:], d_row[:], AF.Square)
            # min over g (stride-64 axis innermost)
            mn_row = ppool.tile([1, SLAB], F32, tag="mn_row")
            sq_kg = bass.AP(sq_row[:].tensor, sq_row[:].offset,
                            [list(sq_row[:].ap[0]), [1, SLAB], [SLAB, NG]])
            nc.vector.tensor_reduce(mn_row[:], sq_kg,
                                    axis=mybir.AxisListType.X, op=OP.min)
            dt_row = ppool.tile([1, NG * SLAB], F32, tag="dt_row")
            mn_bc = bass.AP(mn_row[:].tensor, mn_row[:].offset,
                            [list(mn_row[:].ap[0]), [0, NG], [1, SLAB]])
            nc.vector.tensor_tensor(
                dt_row[:].rearrange("p (g k) -> p g k", g=NG),
                sq_row[:].rearrange("p (g k) -> p g k", g=NG),
                mn_bc, OP.subtract)
            w_row = ppool.tile([1, NG * SLAB], F32, tag="w_row")
            nc.scalar.activation(w_row[:], dt_row[:], AF.Exp, scale=K3)
            n_row = ppool.tile([1, SLAB], F32, tag="n_row")
            w_kg = bass.AP(w_row[:].tensor, w_row[:].offset,
                           [list(w_row[:].ap[0]), [1, SLAB], [SLAB, NG]])
            nc.vector.tensor_reduce(n_row[:], w_kg,
                                    axis=mybir.AxisListType.X, op=OP.add)
            ninv_row = ppool.tile([1, SLAB], F32, tag="ninv_row")
            nc.vector.reciprocal(ninv_row[:], n_row[:])
            phi_row = ppool.tile([1, NG * SLAB], F32, tag="phi_row")
            ninv_bc = bass.AP(ninv_row[:].tensor, ninv_row[:].offset,
                              [list(ninv_row[:].ap[0]), [0, NG], [1, SLAB]])
            nc.vector.tensor_tensor(
                phi_row[:].rearrange("p (g k) -> p g k", g=NG),
                w_row[:].rearrange("p (g k) -> p g k", g=NG),
                ninv_bc, OP.mult)

            cc_in = dpool.tile([1, NG * SLAB], F32, tag="ccin")
            cc_out = dpool.tile([NCORES, NG * SLAB], F32, tag="ccout")
            nc.gpsimd.dma_start(cc_in[:], phi_row[:])
            nc.gpsimd.collective_compute(
                "AllGather", OP.bypass, replica_groups=[list(range(NCORES))],
                ins=[cc_in.opt()], outs=[cc_out.opt()],
            )
            # phi_t[p, b*NG+g] = phi_g(atom 128*b + p), two affine halves.
            # Triggered from the SP queue so the CC wait does not stall Pool.
            phi_t = ppool.tile([128, JB * NG], F32, tag="phi_t")
            cc_flat = cc_out[:].rearrange("a b -> (a b)")
            for h in range(2):
                for b in range(JB):
                    src = bass.AP(cc_flat.tensor,
                                  cc_flat.offset + (2 * b + h) * NG * SLAB,
                                  [[1, SLAB], [SLAB, NG]])
                    nc.sync.dma_start(
                        phi_t[h * SLAB:(h + 1) * SLAB,
                              b * NG:(b + 1) * NG], src)

            # ---------- PC path (local phi only) ----------
            PC_ps = psP.tile([SLAB, NZ * NG], F32, tag="PC")
            for a in range(NG):
                phiA_ps = psS.tile([NZ, SLAB], F32, tag="small")
                nc.tensor.matmul(phiA_ps[:], ones1x95[:],
                                 phi_row[:, a * SLAB:(a + 1) * SLAB],
                                 start=True, stop=True)
                PT = spool.tile([NZ, SLAB], BF16, tag="PT")
                nc.vector.tensor_tensor(PT[:], ohZiT_s, phiA_ps[:], OP.mult)
                nc.tensor.matmul(PC_ps[:], PT[:],
                                 C2_s[:, a * NZ * NG:(a + 1) * NZ * NG],
                                 start=(a == 0), stop=False)
                nc.tensor.matmul(
                    PC_ps[:], PT[:],
                    C2_s[:, (NG + a) * NZ * NG:(NG + a + 1) * NZ * NG],
                    start=False, stop=(a == NG - 1))

            # ---------- 6/8 term chains (all funcs in exp_and_others) ----------
            Ared_s, Bred_s = [], []
            for b in range(JB):
                t6h = tpool.tile([128, FREE], BF16, tag="t6h")
                nc.scalar.activation(t6h[:], g6_s[b][:], AF.Tanh, scale=0.5)
                t8h = tpool.tile([128, FREE], F32, tag="t8h")
                nc.scalar.activation(t8h[:], g6_s[b][:], AF.Tanh,
                                     bias=bias_rho[:], scale=float(4.0 / 7.0))
                u6 = tpool.tile([128, FREE], BF16, tag="u6")
                nc.scalar.activation(u6[:], L2_s[b][:], AF.Exp,
                                     bias=bias_ln05[:], scale=-3.0)
                u8 = tpool.tile([128, FREE], BF16, tag="u8")
                nc.scalar.activation(u8[:], L2_s[b][:], AF.Exp,
                                     bias=bias_ln05[:], scale=-4.0)

                term6 = tpool.tile([128, FREE], BF16, tag="term6")
                nc.vector.scalar_tensor_tensor(
                    term6[:], t6h[:], 1.0, u6[:], OP.add, OP.mult)
                Ared = rpool.tile([128, SLAB], F32, tag=f"Ared_{b}")
                nc.vector.tensor_reduce(
                    Ared[:], term6[:].rearrange("p (i s) -> p i s", s=NS),
                    axis=mybir.AxisListType.X, op=OP.add)
                Ared_s.append(Ared)
                term8 = tpool.tile([128, FREE], BF16, tag="term8")
                nc.vector.scalar_tensor_tensor(
                    term8[:], t8h[:], 1.0, u8[:], OP.add, OP.mult)
                Bred = rpool.tile([128, SLAB], F32, tag=f"Bred_{b}")
                nc.vector.tensor_reduce(
                    Bred[:], term8[:].rearrange("p (i s) -> p i s", s=NS),
                    axis=mybir.AxisListType.X, op=OP.add)
                Bred_s.append(Bred)

            # ---------- Q, G, W2 (needs phi_t from the collective) ----------
            W2_ps = psW.tile([SLAB, NZ * NG], F32, tag="W2")
            for b in range(JB):
                Q = spool.tile([128, NZ * NG], BF16, tag="Q")
                for g in range(NG):
                    nc.vector.tensor_scalar(
                        Q[:, g * NZ:(g + 1) * NZ],
                        ohzb_s[:, b * NZ:(b + 1) * NZ],
                        phi_t[:, (b * NG + g):(b * NG + g + 1)], None, OP.mult)
                t1 = spool.tile([128, SLAB], F32, tag="g_t1")
                nc.vector.tensor_scalar(t1[:], Bred_s[b][:],
                                        r2r4c_s[:, b:b + 1], None, OP.mult)
                t2 = spool.tile([128, SLAB], F32, tag="g_t2")
                nc.vector.tensor_tensor(t2[:], t1[:], r2r4i_s[:], OP.mult)
                G = spool.tile([128, SLAB], BF16, tag="G")
                nc.vector.scalar_tensor_tensor(
                    G[:], t2[:], CB8, Ared_s[b][:],
                    OP.mult, OP.add)
                nc.tensor.matmul(W2_ps[:], G[:], Q[:],
                                 start=(b == 0), stop=(b == JB - 1))

            PC_s = spool.tile([SLAB, NZ * NG], F32, tag="PCs")
            nc.vector.tensor_copy(PC_s[:], PC_ps[:])
            scr = spool.tile([SLAB, NZ * NG], F32, tag="scr")
            nc.vector.tensor_tensor(scr[:], W2_ps[:], PC_s[:], OP.mult)
            E_col = ppool.tile([SLAB, 1], F32, tag="Ecol")
            nc.vector.tensor_reduce(E_col[:], scr[:],
                                    axis=mybir.AxisListType.X, op=OP.add)
            nc.sync.dma_start(e_part[:], E_col[:, 0])

    _split_excess_waits(nc)
    return nc


# ----------------------------------------------------------------------
# host side
# ----------------------------------------------------------------------

def _check_separable(c6ab):
    t1 = c6ab[..., 1]
    t2 = c6ab[..., 2]
    g = t1[0, 0, :, 0]
    ok = (np.abs(t1 - g[None, None, :, None]).max() == 0.0
          and np.abs(t2 - g[None, None, None, :]).max() == 0.0
          and (c6ab[..., 0] > 0).all())
    return ok, g.astype(np.float32)


def _host_prep(Z, pos, shift_int, cell, c6ab, r0ab, rcov, r2r4):
    f32 = np.float32
    Zi = np.clip(np.asarray(Z).astype(np.int64), 0, NZ - 1)
    pos_b = (np.asarray(pos, f32) / f32(AUTOANG)).astype(f32)
    cell_b = (np.asarray(cell, f32) / f32(AUTOANG)).astype(f32)
    shifts = (np.asarray(shift_int, f32) @ cell_b).astype(f32)
    rcov_z = np.asarray(rcov, f32)[Zi]
    r2r4_z = np.asarray(r2r4, f32)[Zi]

    ok, g = _check_separable(np.asarray(c6ab, f32))
    if not ok:
        return None

    # geometry safety: all pairs within cutoff, min pair r2 safely positive
    d = pos_b[None, :, None, :] - pos_b[:, None, None, :] + \
        shifts[None, None, :, :]
    r2_full = (d * d).sum(-1)
    self_mask = np.zeros_like(r2_full, bool)
    self_mask[np.arange(N), np.arange(N), NS // 2] = True
    r2_off = r2_full[~self_mask]
    if r2_off.min() < R2MIN_HOST or r2_full.max() > CUTOFF * CUTOFF:
        return None
    if r2_full[self_mask].max() > 1e-6:
        return None   # zero-shift self distance must be ~0

    bf16 = ml_dtypes.bfloat16

    def split3(x):
        x = np.asarray(x, np.float64)
        h = x.astype(bf16)
        r = x - h.astype(np.float64)
        m = r.astype(bf16)
        l = (r - m.astype(np.float64)).astype(bf16)
        return h, m, l

    # r2[j,f] = |p_j|^2 - 2 p_j.y_f + |y_f|^2 + self-offset, one bf16 matmul
    pj2 = (pos_b.astype(np.float64) ** 2).sum(-1)
    Lcat = np.zeros((94, N), bf16)
    ph, pm, pl = split3(pos_b.T)
    p2h, p2m, p2l = split3(pj2)
    onesN = np.ones(N, bf16)
    for dd in range(3):
        base = dd * 8
        Lcat[base + 0] = ph[dd]; Lcat[base + 1] = ph[dd]
        Lcat[base + 2] = pm[dd]; Lcat[base + 3] = pm[dd]
        Lcat[base + 4] = ph[dd]; Lcat[base + 5] = pl[dd]
        Lcat[base + 6] = pm[dd]; Lcat[base + 7] = pl[dd]
    Lcat[24] = p2h; Lcat[25] = p2m; Lcat[26] = p2l
    Lcat[27] = onesN; Lcat[28] = onesN; Lcat[29] = onesN

    oh = np.zeros((N, NZ), f32)
    oh[np.arange(N), Zi] = 1.0
    r0s = np.asarray(r0ab, f32)

    P95F = np.zeros((NZ, P95F_W), f32)
    P95F[:, P95F_COLS["r0abT"][0]:P95F_COLS["r0abT"][1]] = r0s.T
    P95F[:, P95F_COLS["ohZT"][0]:P95F_COLS["ohZT"][1]] = oh.T
    P2F = np.zeros((2, P2F_W), f32)
    P2F[0, :N] = rcov_z
    P2F[1, :N] = 1.0
    R2R4C = np.ascontiguousarray(r2r4_z.reshape(JB, 128).T)

    C2 = np.asarray(c6ab, np.float64)[..., 0].transpose(2, 0, 3, 1).reshape(
        NZ * NG, NZ * NG)
    C2h = C2.astype(bf16)
    C2l = (C2 - C2h.astype(np.float64)).astype(bf16)
    C2cat = np.zeros((NZ, 2 * NG * NZ * NG), bf16)
    for a in range(NG):
        C2cat[:, a * NZ * NG:(a + 1) * NZ * NG] = C2h[a * NZ:(a + 1) * NZ, :]
        C2cat[:, (NG + a) * NZ * NG:(NG + a + 1) * NZ * NG] = \
            C2l[a * NZ:(a + 1) * NZ, :]
    OHZB = np.zeros((128, JB * NZ), bf16)
    for b in range(JB):
        OHZB[:, b * NZ:(b + 1) * NZ] = oh[b * 128:(b + 1) * 128, :]

    GROW = np.repeat(g, SLAB).astype(f32)   # [320] g-major

    y_all = pos_b[:, None, :] - shifts[None, :, :]          # [N, S, 3]
    in_maps = []
    for c in range(NCORES):
        isl = slice(c * SLAB, (c + 1) * SLAB)
        y = y_all[isl].reshape(FREE, 3).astype(f32)
        y2 = (y.astype(np.float64) ** 2).sum(-1)
        q = -2.0 * y.astype(np.float64)
        Rcat = np.zeros((94, FREE), bf16)
        qh, qm, ql = split3(q.T)
        y2h, y2m, y2l = split3(y2)
        onesF = np.ones(FREE, bf16)
        for dd in range(3):
            base = dd * 8
            Rcat[base + 0] = qh[dd]; Rcat[base + 1] = qm[dd]
            Rcat[base + 2] = qh[dd]; Rcat[base + 3] = qm[dd]
            Rcat[base + 4] = ql[dd]; Rcat[base + 5] = qh[dd]
            Rcat[base + 6] = ql[dd]; Rcat[base + 7] = qm[dd]
        Rcat[24] = onesF; Rcat[25] = onesF; Rcat[26] = onesF
        Rcat[27] = y2h; Rcat[28] = y2m; Rcat[29] = y2l
        Lc = Lcat.copy()
        Lc[30 + np.arange(SLAB), c * SLAB + np.arange(SLAB)] = bf16(SELF_R2)
        Rcat[30 + np.arange(SLAB), np.arange(SLAB) * NS + (NS // 2)] = bf16(1.0)
        LRcat = np.concatenate([Lc, Rcat], axis=1)

        P95c = P95F.copy()
        P95c[:, P95F_COLS["ohZiT"][0]:P95F_COLS["ohZiT"][1]] = oh[isl].T
        P2c = P2F.copy()
        P2c[0, N:] = 1.0
        P2c[1, N:] = rcov_z[isl]
        P1c = np.zeros((1, P1F_W), f32)
        P1c[0, :SLAB] = r2r4_z[isl]
        P1c[0, SLAB:] = GROW

        per = dict(LRcat=LRcat, P95F=P95c, P2F=P2c, P1F=P1c,
                   R2R4C=R2R4C, C2cat=C2cat, OHZB=OHZB)
        in_maps.append(per)
    return in_maps


def _numpy_fallback(Z, pos, shift_int, cell, c6ab, r0ab, rcov, r2r4):
    """Exact reference math in numpy (f32), used only when the fast-path
    assumptions do not hold."""
    f32 = np.float32
    Zi = np.asarray(Z).astype(np.int64)
    pos_b = np.asarray(pos, f32) / f32(AUTOANG)
    cell_b = np.asarray(cell, f32) / f32(AUTOANG)
    shifts = np.asarray(shift_int, f32) @ cell_b
    d = pos_b[None, :, None, :] - pos_b[:, None, None, :] + shifts[None, None, :, :]
    r2 = (d * d).sum(-1)
    mask = r2 > 1e-8
    r = np.sqrt(np.where(mask, r2, 1.0))
    in_cut = mask & (r <= CUTOFF)
    rcov_z = np.asarray(rcov, f32)[Zi]
    rco = rcov_z[:, None] + rcov_z[None, :]
    dmp = 1.0 / (1.0 + np.exp(-K1 * (rco[:, :, None] / r - 1.0)))
    cn = np.where(mask & (r <= CNTHR), dmp, 0.0).sum(axis=(1, 2))
    tbl = np.asarray(c6ab, f32)[Zi[:, None], Zi[None, :]]
    c6r = tbl[..., 0]
    valid = c6r > 0.0
    dcn = (cn[:, None, None, None] - tbl[..., 1]) ** 2 + \
          (cn[None, :, None, None] - tbl[..., 2]) ** 2
    dmin = np.where(valid, dcn, 1e10).min(axis=(-2, -1), keepdims=True)
    w = np.where(valid, np.exp(K3 * (dcn - dmin)), 0.0)
    c6 = (c6r * w).sum((-2, -1)) / np.maximum(w.sum((-2, -1)), 1e-20)
    r2r4_z = np.asarray(r2r4, f32)[Zi]
    c8 = 3.0 * c6 * r2r4_z[:, None] * r2r4_z[None, :]
    r0 = np.asarray(r0ab, f32)[Zi[:, None], Zi[None, :]]
    r6 = np.where(mask, r2, 1.0) ** 3
    r8 = r6 * np.where(mask, r2, 1.0)
    t6 = (r / (RS6 * r0[:, :, None])) ** (-ALP)
    t8 = (r / (RS18 * r0[:, :, None])) ** (-(ALP + 2.0))
    e6 = S6 * c6[:, :, None] / r6 / (1.0 + 6.0 * t6)
    e8 = S18 * c8[:, :, None] / r8 / (1.0 + 6.0 * t8)
    E = -0.5 * np.where(in_cut, e6 + e8, 0.0).sum(dtype=np.float64)
    return np.asarray(np.float32(AUTOEV * E))


_PROGRAM_CACHE = {}


def kernel(**inputs) -> np.ndarray:
    inputs = {k: np.asarray(v) for k, v in inputs.items()}
    shapes_ok = (inputs["pos"].shape == (N, 3)
                 and inputs["shift_int"].shape == (NS, 3)
                 and inputs["c6ab"].shape == (NZ, NZ, NG, NG, 3))
    in_maps = _host_prep(**inputs) if shapes_ok else None
    if in_maps is None:
        return _numpy_fallback(**inputs)

    if "nc" not in _PROGRAM_CACHE:
        _PROGRAM_CACHE["nc"] = build_program()
    nc = _PROGRAM_CACHE["nc"]

    trace = bool(os.environ.get("D3_TRACE"))
    res = run_bass_kernel_spmd(nc, in_maps, list(range(NCORES)), trace=trace)
    _PROGRAM_CACHE["last_exec_time_ns"] = res.exec_time_ns
    _PROGRAM_CACHE["last_results"] = res
    e = np.zeros((), np.float64)
    for c in range(NCORES):
        e += res.results[c]["e_part"].astype(np.float64).sum()
    out = np.float32(-0.5 * AUTOEV * S6 * e)
    return np.asarray(out)


if __name__ == "__main__":
    nc = build_program()
    print("program built:",
          sum(len(b.instructions) for f in nc.m.functions for b in f.blocks),
          "instructions")
